# revision 3
# baseline (speedup 1.0000x reference)
"""Trainium2 Bass kernel for nn_MultiHeadAttention_3796751090171 (sparse_attention).

Head-parallel SPMD across 8 NeuronCores: core c computes head c.

Math per head h:
  Q = q_feat @ Wq[:, h*64:(h+1)*64] + bq_h          [N, 64]
  K = k_feat @ Wk_h + bk_h                           [N, 64]
  V = v_feat @ Wv_h + bv_h                           [N, 64]
  S = Q @ K.T / 8 + pos_enc[h]                       [N, N]
  S masked to -inf where q_batch[i] != k_batch[j]
  P = softmax(S, axis=-1)
  out_partial = (P @ V) @ Wo[h*64:(h+1)*64, :] + bo/8
  out = sum_h out_partial  (on-device ReduceScatter)

Sparsity: q_batch/k_batch are SORTED, so the mask is block-diagonal over
(q-batch-range x k-batch-range) blocks; we only compute those blocks and skip
masking entirely (a k-chunk never crosses a batch boundary).

Layout: everything is computed transposed (scoresT [k, q]) so that
  - scoresT tile = KT_chunk.T @ QT        (no transposes needed)
  - probsT feeds h-matmul directly: h_unnormT = [V | 1].T @ expT; the appended
    ones column makes row 64 of the PSUM accumulator the softmax denominator Z.
  - out rows are scaled by 1/Z at the final PSUM->SBUF copy (per-partition scalar)

No max-subtraction in softmax: scores are O(10) so exp is safe in fp32, and
masked blocks are simply never computed (exp -> exact 0, same as reference's
exp(-1e9 - max) -> 0).
"""

import functools
import math

import numpy as np
import ml_dtypes

import concourse.bass as bass
import concourse.tile as tile
from concourse import bacc, mybir
from concourse.bass_utils import run_bass_kernel_spmd
from concourse.masks import make_identity

N = 3072
QD = 512
OD = 512
H = 8
D = 64
B = 8
NCORES = 8
SCALE = math.sqrt(D)

F32 = mybir.dt.float32
BF16 = mybir.dt.bfloat16
BF16_NP = ml_dtypes.bfloat16

# test.py can flip these to get a profile
TRACE = False
LAST_RESULTS = None


def _plan(q_batch, k_batch):
    """Batch block boundaries from the sorted batch-id vectors."""
    qb = np.asarray(q_batch).astype(np.int64)
    kb = np.asarray(k_batch).astype(np.int64)
    qbound = np.searchsorted(qb, np.arange(B + 1))
    kbound = np.searchsorted(kb, np.arange(B + 1))
    batches = []
    degenerate = False
    for b in range(B):
        q0, q1 = int(qbound[b]), int(qbound[b + 1])
        k0, k1 = int(kbound[b]), int(kbound[b + 1])
        if q1 > q0 and k1 > k0:
            batches.append((q0, q1, k0, k1))
        elif q1 > q0 and k1 == k0:
            # rows with no visible keys: reference gives uniform attention
            # over ALL keys; handled by numpy fallback (never happens with
            # realistic random batch vectors)
            degenerate = True
    return tuple(batches), degenerate


def _chunks(lo, hi, step):
    return [(o, min(step, hi - o)) for o in range(lo, hi, step)]


@functools.lru_cache(maxsize=8)
def _build(batches, has_bq, has_bk, has_bv, has_bo):
    nc = bacc.Bacc("TRN2", target_bir_lowering=False, debug=False,
                   num_devices=NCORES)

    # ---- DRAM parameters (per-core values supplied via in_maps) ----
    qfT_d = nc.dram_tensor("qfT", [QD, N], BF16, kind="ExternalInput")
    kfT_d = nc.dram_tensor("kfT", [QD, N], BF16, kind="ExternalInput")
    vfT_d = nc.dram_tensor("vfT", [QD, N], BF16, kind="ExternalInput")
    posT_d = nc.dram_tensor("posT", [N, N], BF16, kind="ExternalInput")
    wq_d = nc.dram_tensor("wq", [QD, D], BF16, kind="ExternalInput")
    wk_d = nc.dram_tensor("wk", [QD, D], BF16, kind="ExternalInput")
    wv_d = nc.dram_tensor("wv", [QD, D], BF16, kind="ExternalInput")
    wo_d = nc.dram_tensor("wo", [D, OD], BF16, kind="ExternalInput")
    bq_d = nc.dram_tensor("bq", [1, D], BF16, kind="ExternalInput") if has_bq else None
    bk_d = nc.dram_tensor("bk", [1, D], BF16, kind="ExternalInput") if has_bk else None
    bv_d = nc.dram_tensor("bv", [1, D], BF16, kind="ExternalInput") if has_bv else None
    bo_d = nc.dram_tensor("bo8", [1, OD], BF16, kind="ExternalInput") if has_bo else None
    out_d = nc.dram_tensor("out", [N // NCORES, OD], F32, kind="ExternalOutput")

    # global k-chunk list, aligned to batch boundaries (never crosses one)
    kchunk_list = []   # (koff, klen)
    batch_kchunks = []  # per batch: list of global chunk indices
    for (q0, q1, k0, k1) in batches:
        idxs = []
        for (koff, klen) in _chunks(k0, k1, 128):
            idxs.append(len(kchunk_list))
            kchunk_list.append((koff, klen))
        batch_kchunks.append(idxs)
    nch = len(kchunk_list)

    KT_T = 4  # 512 contraction split in 4 k-tiles of 128
    NQC = N // 512  # q chunks for projections
    NOC = N // 128  # q chunks for out-proj

    with tile.TileContext(nc) as tc:
        with (
            tc.tile_pool(name="consts", bufs=1) as consts,
            tc.tile_pool(name="feat", bufs=2) as featp,
            tc.tile_pool(name="persist", bufs=1) as pers,
            tc.tile_pool(name="pos", bufs=6) as posp,
            tc.tile_pool(name="expp", bufs=6) as expp,
            tc.tile_pool(name="outp", bufs=3) as outp,
            tc.tile_pool(name="small", bufs=2) as smallp,
            tc.tile_pool(name="ps_s", bufs=3, space="PSUM") as ps_s,
            tc.tile_pool(name="ps_h", bufs=2, space="PSUM") as ps_h,
            tc.tile_pool(name="ps_p", bufs=2, space="PSUM") as ps_p,
            tc.tile_pool(name="dram", bufs=1, space="DRAM") as dramp,
        ):
            # ---------------- constants ----------------
            ones = consts.tile([1, N], BF16)
            nc.vector.memset(ones, 1.0)
            ident = consts.tile([24, 24], F32)
            make_identity(nc, ident)

            wq_sb = consts.tile([128, KT_T, D], BF16)
            wk_sb = consts.tile([128, KT_T, D], BF16)
            wv_sb = consts.tile([128, KT_T, D], BF16)
            nc.sync.dma_start(out=wq_sb, in_=wq_d.ap().rearrange("(t p) d -> p t d", p=128))
            nc.sync.dma_start(out=wk_sb, in_=wk_d.ap().rearrange("(t p) d -> p t d", p=128))
            nc.sync.dma_start(out=wv_sb, in_=wv_d.ap().rearrange("(t p) d -> p t d", p=128))
            wo_sb = consts.tile([D, OD], BF16)
            nc.sync.dma_start(out=wo_sb, in_=wo_d[:, :])
            bias_sb = {}
            for nm, dd in (("bq", bq_d), ("bk", bk_d), ("bv", bv_d)):
                if dd is not None:
                    t = consts.tile([1, D], BF16, tag=f"bias_{nm}")
                    nc.sync.dma_start(out=t, in_=dd[:, :])
                    bias_sb[nm] = t
            if bo_d is not None:
                bo_sb = consts.tile([1, OD], BF16)
                nc.sync.dma_start(out=bo_sb, in_=bo_d[:, :])

            # persistent intermediates
            QT_sb = pers.tile([D, N], BF16)    # Q^T/8 with bias folded
            KT_sb = pers.tile([D, N], BF16)
            V_sb = pers.tile([128, nch, D + 1], BF16)  # [k, chunk, d | ones]
            hT_sb = pers.tile([D, N], BF16)    # unnormalized h^T
            Zrow_sb = pers.tile([1, N], F32)   # softmax denominators (q-major)
            Zres_sb = pers.tile([24, 128], F32)
            recipZ_sb = pers.tile([128, 24], F32)

            # ---------------- projections ----------------
            def project_T(feat_d, w_sb, bias, dst):
                # dst[d, q] = (w.T @ featT)[d, q] (+ bias[d] via rank-1 mm)
                f_sb = featp.tile([128, KT_T, N], BF16, tag="feat")
                for t in range(KT_T):
                    nc.sync.dma_start(
                        out=f_sb[:, t, :],
                        in_=feat_d.ap().rearrange("(t p) n -> t p n", p=128)[t],
                    )
                for qc in range(NQC):
                    qsl = slice(qc * 512, (qc + 1) * 512)
                    psum = ps_p.tile([128, 512], F32, tag="psp")
                    for t in range(KT_T):
                        nc.tensor.matmul(psum[0:D, :], w_sb[:, t, :],
                                         f_sb[:, t, qsl],
                                         start=(t == 0), stop=(t == KT_T - 1 and bias is None))
                    if bias is not None:
                        nc.tensor.matmul(psum[0:D, :], bias, ones[:, qsl],
                                         start=False, stop=True)
                    nc.any.tensor_copy(dst[:, qsl], psum[0:D, :])
                return f_sb

            project_T(qfT_d, wq_sb, bias_sb.get("bq"), QT_sb)
            project_T(kfT_d, wk_sb, bias_sb.get("bk"), KT_sb)

            # V projection into batch-aligned k-chunks, with ones column
            vf_sb = featp.tile([128, KT_T, N], BF16, tag="feat")
            for t in range(KT_T):
                nc.sync.dma_start(
                    out=vf_sb[:, t, :],
                    in_=vfT_d.ap().rearrange("(t p) n -> t p n", p=128)[t],
                )
            nc.vector.memset(V_sb[:, :, D], 1.0)
            for j, (koff, klen) in enumerate(kchunk_list):
                ksl = slice(koff, koff + klen)
                psum = ps_p.tile([128, 512], F32, tag="psp")
                bv = bias_sb.get("bv")
                for t in range(KT_T):
                    nc.tensor.matmul(psum[0:klen, 0:D], vf_sb[:, t, ksl],
                                     wv_sb[:, t, :],
                                     start=(t == 0), stop=(t == KT_T - 1 and bv is None))
                if bv is not None:
                    nc.tensor.matmul(psum[0:klen, 0:D], ones[:, 0:klen], bv,
                                     start=False, stop=True)
                nc.any.tensor_copy(V_sb[0:klen, j, 0:D], psum[0:klen, 0:D])

            # ---------------- attention (block-diagonal) ----------------
            for bi, (q0, q1, k0, k1) in enumerate(batches):
                for (qoff, qw) in _chunks(q0, q1, 512):
                    qsl = slice(qoff, qoff + qw)
                    psum_h = ps_h.tile([D + 1, 512], F32, tag="psh")
                    idxs = batch_kchunks[bi]
                    for ii, j in enumerate(idxs):
                        koff, klen = kchunk_list[j]
                        ksl = slice(koff, koff + klen)
                        ps = ps_s.tile([128, 512], F32, tag="pss")
                        nc.tensor.matmul(ps[0:klen, 0:qw], KT_sb[:, ksl],
                                         QT_sb[:, qsl], start=True, stop=True)
                        pos = posp.tile([128, 512], BF16, tag="pos")
                        nc.sync.dma_start(out=pos[0:klen, 0:qw],
                                          in_=posT_d[ksl, qsl])
                        nc.vector.tensor_add(ps[0:klen, 0:qw], ps[0:klen, 0:qw],
                                             pos[0:klen, 0:qw])
                        expt = expp.tile([128, 512], BF16, tag="expt")
                        nc.scalar.activation(expt[0:klen, 0:qw], ps[0:klen, 0:qw],
                                             mybir.ActivationFunctionType.Exp)
                        nc.tensor.matmul(psum_h[:, 0:qw], V_sb[0:klen, j, :],
                                         expt[0:klen, 0:qw],
                                         start=(ii == 0), stop=(ii == len(idxs) - 1))
                    nc.any.tensor_copy(hT_sb[:, qsl], psum_h[0:D, 0:qw])
                    nc.any.tensor_copy(Zrow_sb[:, qsl], psum_h[D:D + 1, 0:qw])

            # ---------------- Z transpose + reciprocal ----------------
            zb = dramp.tile([1, N], F32)
            nc.sync.dma_start(out=zb[:, :], in_=Zrow_sb[0:1, :])
            nc.sync.dma_start(out=Zres_sb[:, :],
                              in_=zb[0:1, :].rearrange("p (a b) -> (p a) b", b=128))
            ps_zt = ps_p.tile([128, 512], F32, tag="psp")
            nc.tensor.transpose(ps_zt[:, 0:24], Zres_sb[:, :], ident[:, :])
            nc.vector.reciprocal(recipZ_sb[:, :], ps_zt[:, 0:24])

            # ---------------- output projection + ReduceScatter ----------------
            partial = dramp.tile([N, OD], F32)
            for c in range(NOC):
                csl = slice(c * 128, (c + 1) * 128)
                psum = ps_p.tile([128, 512], F32, tag="psp")
                nc.tensor.matmul(psum[:, :], hT_sb[:, csl], wo_sb[:, :],
                                 start=True, stop=(bo_d is None))
                if bo_d is not None:
                    nc.tensor.matmul(psum[:, :], ones[:, 0:128], bo_sb[:, :],
                                     start=False, stop=True)
                o_sb = outp.tile([128, OD], F32, tag="osb")
                nc.vector.tensor_scalar(o_sb[:, :], psum[:, :],
                                        recipZ_sb[:, c:c + 1], None,
                                        op0=mybir.AluOpType.mult)
                nc.sync.dma_start(out=partial[csl, :], in_=o_sb[:, :])

            rs_out = dramp.tile([N // NCORES, OD], F32)
            nc.gpsimd.collective_compute(
                "ReduceScatter",
                mybir.AluOpType.add,
                replica_groups=[list(range(NCORES))],
                ins=[partial.opt()],
                outs=[rs_out.opt()],
            )
            nc.sync.dma_start(out=out_d[:, :], in_=rs_out[:, :])

    nc.compile()
    return nc


def _kernel_numpy(q_feat, k_feat, v_feat, pos_enc, Wq, bq, Wk, bk, Wv, bv,
                  Wo, bo, q_batch, k_batch):
    """Host fallback (degenerate batch layouts only) + debugging aid."""
    Q = (q_feat @ Wq + bq).reshape(N, H, D).transpose(1, 0, 2)
    K = (k_feat @ Wk + bk).reshape(N, H, D).transpose(1, 0, 2)
    V = (v_feat @ Wv + bv).reshape(N, H, D).transpose(1, 0, 2)
    scores = np.einsum("hnd,hmd->hnm", Q, K) / SCALE + pos_enc
    mask = q_batch[:, None] != k_batch[None, :]
    scores = np.where(mask[None], np.float32(-1e9), scores)
    scores = scores - scores.max(-1, keepdims=True)
    e = np.exp(scores)
    probs = e / e.sum(-1, keepdims=True)
    h = np.einsum("hnm,hmd->hnd", probs, V)
    h = h.transpose(1, 0, 2).reshape(N, OD)
    return (h @ Wo + bo).astype(np.float32)


def kernel(q_feat, k_feat, v_feat, pos_enc, Wq, bq, Wk, bk, Wv, bv, Wo, bo,
           q_batch, k_batch):
    global LAST_RESULTS
    args = dict(q_feat=np.asarray(q_feat, np.float32),
                k_feat=np.asarray(k_feat, np.float32),
                v_feat=np.asarray(v_feat, np.float32),
                pos_enc=np.asarray(pos_enc, np.float32),
                Wq=np.asarray(Wq, np.float32), bq=np.asarray(bq, np.float32),
                Wk=np.asarray(Wk, np.float32), bk=np.asarray(bk, np.float32),
                Wv=np.asarray(Wv, np.float32), bv=np.asarray(bv, np.float32),
                Wo=np.asarray(Wo, np.float32), bo=np.asarray(bo, np.float32),
                q_batch=np.asarray(q_batch), k_batch=np.asarray(k_batch))

    batches, degenerate = _plan(args["q_batch"], args["k_batch"])
    if degenerate or not batches:
        return _kernel_numpy(**args)

    has_bq = bool(np.any(args["bq"]))
    has_bk = bool(np.any(args["bk"]))
    has_bv = bool(np.any(args["bv"]))
    has_bo = bool(np.any(args["bo"]))

    nc = _build(batches, has_bq, has_bk, has_bv, has_bo)

    # ---- host-side sharding / layout prep ----
    qfT = np.ascontiguousarray(args["q_feat"].T).astype(BF16_NP)
    kfT = np.ascontiguousarray(args["k_feat"].T).astype(BF16_NP)
    vfT = np.ascontiguousarray(args["v_feat"].T).astype(BF16_NP)

    in_maps = []
    for c in range(NCORES):
        hs = slice(c * D, (c + 1) * D)
        m = {
            "qfT": qfT, "kfT": kfT, "vfT": vfT,
            "posT": np.ascontiguousarray(
                args["pos_enc"][c].astype(BF16_NP).T),
            "wq": (args["Wq"][:, hs] / SCALE).astype(BF16_NP),
            "wk": args["Wk"][:, hs].astype(BF16_NP),
            "wv": args["Wv"][:, hs].astype(BF16_NP),
            "wo": np.ascontiguousarray(args["Wo"][hs, :]).astype(BF16_NP),
        }
        if has_bq:
            m["bq"] = (args["bq"][hs] / SCALE).astype(BF16_NP).reshape(1, D)
        if has_bk:
            m["bk"] = args["bk"][hs].astype(BF16_NP).reshape(1, D)
        if has_bv:
            m["bv"] = args["bv"][hs].astype(BF16_NP).reshape(1, D)
        if has_bo:
            m["bo8"] = (args["bo"] / NCORES).astype(BF16_NP).reshape(1, OD)
        in_maps.append(m)

    res = run_bass_kernel_spmd(nc, in_maps, core_ids=list(range(NCORES)),
                               trace=TRACE)
    LAST_RESULTS = res
    return np.concatenate([res.results[i]["out"] for i in range(NCORES)], axis=0)
